# revision 59
# baseline (speedup 1.0000x reference)
"""Trainium2 Bass kernel for nn_Attn_90744069030604 (dense transformer block
with linear attention), distributed over 8 NeuronCores.

Sharding: token-parallel. The 16384 tokens (B=4 x S=4096) are split into 8
contiguous blocks of 2048; core c gets batch c//2, sequence half c%2. All
matmuls (qkv / to_out / MLP) are token-local. The only cross-core coupling is
the linear-attention context ctx = softmax_S(k)^T v and its normalizer Z_k,
both reductions over the full sequence of one batch: each core computes
partials over its half-sequence and a pairwise (cores 2b, 2b+1) AllReduce of
~266KB combines them, overlapped with the q-projection matmuls.

v2: fp8(e4m3) DoubleRow matmuls for all five big GEMMs (qkv, to_out, W1, W2)
at 2x PE throughput; weights are scaled x64 into fp8 and descaled via the
ACT-engine activation `scale`. x stays bf16 end-to-end. Positional-encoding /
LN-mean terms are folded into precomputed tensors added in-PSUM via identity
matmuls (PE) instead of DVE adds. exp(q) is kept unnormalized in fp8; 1/Zq is
applied to the attention PSUM output (pa*pb) per head-pair. Reciprocals use
the fast approx DVE op. LN gamma/beta and biases are host-folded as before.
"""

import math
import os
import sys
from contextlib import ExitStack

sys.path.insert(0, "/opt/trn_rl_repo")

import numpy as np
import ml_dtypes

import concourse.bass as bass  # noqa: F401
import concourse.bacc as bacc
import concourse.mybir as mybir
import concourse.tile as tile
from concourse.bass_utils import run_bass_kernel_spmd

F32 = mybir.dt.float32
BF = mybir.dt.bfloat16
F8 = mybir.dt.float8e4
AF = mybir.ActivationFunctionType
ALU = mybir.AluOpType
DR = mybir.MatmulPerfMode.DoubleRow

N_CORES = 8
B, S, D = 4, 4096, 1024
H, DH = 16, 64
T = (B * S) // N_CORES        # 2048 tokens per core
TJ = 512                      # token tile (free dim)
NT = T // TJ                  # 4 token tiles
ND = D // 128                 # 8 feature chunks of 128
NP = ND // 2                  # 4 feature chunk PAIRS (DoubleRow)
EPS = 1e-6
WS = 64.0                     # fp8 weight scale (power of 2)
IWS = 1.0 / WS

BFNP = ml_dtypes.bfloat16
F8NP = ml_dtypes.float8_e4m3  # TRN e4m3 (max +-240)


def _bf(a):
    return np.ascontiguousarray(np.asarray(a, np.float32)).astype(BFNP)


def _f8(a):
    return np.ascontiguousarray(np.asarray(a, np.float32)).astype(F8NP)


def _pack_pairs(w, cols):
    """[1024, cols] -> [128, NP*2*cols] fp8 pair-packed for DoubleRow."""
    w = np.asarray(w, np.float32).reshape(NP, 2, 128, cols)
    w = w.transpose(2, 0, 1, 3).reshape(128, NP * 2 * cols)
    return _f8(w)


def build_nc():
    nc = bacc.Bacc("TRN2", target_bir_lowering=False, debug=False,
                   num_devices=N_CORES)

    # ---- DRAM I/O ----
    d_xbf = nc.dram_tensor("xbf", [D, T], BF, kind="ExternalInput")
    d_r1 = nc.dram_tensor("r1_row", [1, T], BF, kind="ExternalInput")
    d_wq8 = nc.dram_tensor("wq8", [128, NP * 2 * D], F8, kind="ExternalInput")
    d_wkv8 = nc.dram_tensor("wkv8", [128, NP * 2 * 2 * D], F8,
                            kind="ExternalInput")
    d_wow28 = nc.dram_tensor("wow28", [128, NP * 2 * 2 * D], F8,
                             kind="ExternalInput")
    d_w18 = nc.dram_tensor("w18", [128, NP * 2 * D], F8, kind="ExternalInput")
    d_pkv64 = nc.dram_tensor("pkv64", [T, 2 * D], F8, kind="ExternalInput")
    d_pq64 = nc.dram_tensor("pq64p", [128, NT * ND * TJ], F8,
                            kind="ExternalInput")
    d_ident = nc.dram_tensor("identd", [128, 3 * 128], BF,
                             kind="ExternalInput")
    d_mask8 = nc.dram_tensor("mask8", [128, NP * 2 * H], F8,
                             kind="ExternalInput")
    d_sel2 = nc.dram_tensor("sel2", [H, D], BF, kind="ExternalInput")
    d_csw1n = nc.dram_tensor("csw1n", [1, D], BF, kind="ExternalInput")
    d_boc = nc.dram_tensor("boc", [128, ND], F32, kind="ExternalInput")
    d_b1c = nc.dram_tensor("b1c", [128, ND], F32, kind="ExternalInput")
    d_b2c = nc.dram_tensor("b2c", [128, ND], F32, kind="ExternalInput")
    d_out = nc.dram_tensor("outT", [D, T], F32, kind="ExternalOutput")

    with tile.TileContext(nc) as tc, ExitStack() as ctx:
        const = ctx.enter_context(tc.tile_pool(name="const", bufs=1))
        wpool = ctx.enter_context(tc.tile_pool(name="w", bufs=1))
        xfp = ctx.enter_context(tc.tile_pool(name="xfp", bufs=3))
        pep = ctx.enter_context(tc.tile_pool(name="pep", bufs=2))
        work = ctx.enter_context(tc.tile_pool(name="work", bufs=3))
        rot = ctx.enter_context(tc.tile_pool(name="rot", bufs=1))
        ps_mm = ctx.enter_context(tc.tile_pool(name="psm", bufs=3, space="PSUM"))
        ps_st = ctx.enter_context(tc.tile_pool(name="pst", bufs=3, space="PSUM"))
        ps_cx = ctx.enter_context(tc.tile_pool(name="psc", bufs=2, space="PSUM"))
        dram = ctx.enter_context(tc.tile_pool(name="dram", bufs=1, space="DRAM"))

        _cms = {}

        def open_pool(key, **kw):
            cm = tc.tile_pool(name=key, bufs=1, **kw)
            _cms[key] = cm
            return cm.__enter__()

        def close_pool(key):
            _cms.pop(key).__exit__(None, None, None)

        zp_z = open_pool("zp_z", side="left")
        zp_kv = open_pool("zp_kv", side="left")

        # ---- constants ----
        one_row = const.tile([1, 128], BF, tag="one_row")
        nc.vector.memset(one_row[:], 1.0)
        oneK = const.tile([128, 1], BF, tag="oneK")
        nc.vector.memset(oneK[:], 1.0 / 1024.0)
        eps_col = const.tile([128, 1], F32, tag="eps_col")
        nc.vector.memset(eps_col[:], EPS)
        ident = const.tile([128, 3 * 128], BF, tag="ident")
        nc.sync.dma_start(ident[:], d_ident[:])
        I1 = ident[:, 0:128]
        I4k = ident[:, 128:256]
        I64 = ident[:, 256:384]

        z8_t, eq8_t, ek8_t, vt8_t, x2_t, zqb_t = {}, {}, {}, {}, {}, {}

        # ---- bulk input loads; x split in half-T tiles so tau-0/1 work
        # starts after 2MB instead of 4MB ----
        r1t = const.tile([1, T], BF, tag="r1t")
        nc.sync.dma_start(r1t[:], d_r1[:])
        boc_t = const.tile([128, ND], F32, tag="boc")
        nc.sync.dma_start(boc_t[:], d_boc[:])
        b1c_t = const.tile([128, ND], F32, tag="b1c")
        nc.sync.dma_start(b1c_t[:], d_b1c[:])
        b2c_t = const.tile([128, ND], F32, tag="b2c")
        nc.sync.dma_start(b2c_t[:], d_b2c[:])
        csw1n_t = const.tile([1, D], BF, tag="csw1n")
        nc.sync.dma_start(csw1n_t[:], d_csw1n[:])
        HT = T // 2
        xbf_h = [[], []]
        for half in range(2):
            for k in range(ND):
                xt = xfp.tile([128, HT], BF, tag=f"xbf{half}_{k}", bufs=1,
                              name=f"xbf{half}_{k}")
                nc.sync.dma_start(
                    xt[:], d_xbf[128 * k:128 * (k + 1),
                                 HT * half:HT * (half + 1)])
                xbf_h[half].append(xt)

        def xbf_sl(k, tau):
            return xbf_h[tau // 2][k][:, TJ * (tau % 2):TJ * (tau % 2 + 1)]

        # ======== Phase A: LN1 scale (z8 = bf16(x) * r, fp8 out) ========
        def emit_ln1(tau):
            p_r = ps_mm.tile([128, TJ], F32, tag="mm", name="p_r")
            nc.tensor.matmul(p_r[:], one_row[:],
                             r1t[0:1, TJ * tau:TJ * (tau + 1)],
                             start=True, stop=True)
            rbc = work.tile([128, TJ], BF, tag="rbc", bufs=2, name="rbc")
            nc.scalar.copy(rbc[:], p_r[:])
            for i in range(NP):
                z8 = zp_z.tile([128, 2 * TJ], F8, tag=f"z_{i}_{tau}",
                               name=f"z_{i}_{tau}")
                z8_t[(i, tau)] = z8
                for j2 in range(2):
                    k = 2 * i + j2
                    nc.vector.tensor_tensor(
                        z8[:, TJ * j2:TJ * (j2 + 1)],
                        xbf_sl(k, tau), rbc[:], ALU.mult)

        emit_ln1(0)
        wkv8_t = []
        for u in range(NP):
            wt = wpool.tile([128, 2 * 2 * D], F8, tag=f"wbig{u}",
                            name=f"wkv8_{u}")
            nc.sync.dma_start(
                wt[:], d_wkv8[:, 2 * 2 * D * u:2 * 2 * D * (u + 1)])
            wkv8_t.append(wt)
        emit_ln1(1)

        # ======== Phase B: k/v projections + ctx rounds ========
        ctx_acc = zp_kv.tile([128, 8 * 65], F32, tag="ctx_acc")
        ctx_all = zp_kv.tile([128, 8 * 65], F32, tag="ctx_all")

        def emit_ctx_round(rnd):
            # two heads per matmul: stationary = ek cols of both heads of
            # pair i ([128, 128]); moving = both heads' v (+ones) columns
            # ([128, 2, 65]); diagonal blocks of the [128, 130] PSUM hold the
            # per-head results.
            for i in range(8):  # head pairs
                pc = ps_cx.tile([128, 2 * 65], F32, tag="ctx",
                                name=f"pc{rnd}_{i}")
                for ui, u in enumerate(range(4 * rnd, 4 * rnd + 4)):
                    for j in range(2):
                        ekv = ek8_t[u][:].rearrange(
                            "p (j n) -> p j n", n=D)[
                                :, j, 128 * i:128 * (i + 1)]
                        vtv = vt8_t[u][:].rearrange(
                            "p (j q c) -> p j q c", j=2, c=65)[
                                :, j, 2 * i:2 * i + 2, :]
                        nc.tensor.matmul(
                            pc[:], ekv, vtv,
                            start=(ui == 0 and j == 0),
                            stop=(ui == 3 and j == 1))
                for par in range(2):
                    pcs = pc[64 * par:64 * (par + 1),
                             65 * par:65 * (par + 1)]
                    acs = ctx_acc[64 * par:64 * (par + 1),
                                  65 * i:65 * (i + 1)]
                    if rnd == 0:
                        nc.scalar.copy(acs, pcs)
                    else:
                        nc.vector.tensor_tensor(
                            ctx_all[64 * par:64 * (par + 1),
                                    65 * i:65 * (i + 1)], pcs, acs, ALU.add)

        for tau in range(NT):
            if tau + 2 < NT:
                emit_ln1(tau + 2)
            if tau == 3:
                # phase-C weights ride late phase-B DMA slack, ahead of the
                # AllReduce's transfers
                sel2_t = const.tile([H, D], BF, tag="sel2")
                nc.sync.dma_start(sel2_t[:], d_sel2[:])
                mask8_t = []
                for i in range(NP):
                    mt = const.tile([128, 2 * H], F8, tag=f"mask{i}",
                                    name=f"mask{i}")
                    nc.sync.dma_start(
                        mt[:], d_mask8[:, 2 * H * i:2 * H * (i + 1)])
                    mask8_t.append(mt)
                wq8_t = []
                for u2 in range(NP):
                    wt = wpool.tile([128, 2 * D], F8, tag=f"wsm{u2}",
                                    name=f"wq8_{u2}")
                    nc.sync.dma_start(
                        wt[:], d_wq8[:, 2 * D * u2:2 * D * (u2 + 1)])
                    wq8_t.append(wt)
            for tl in range(NT):
                tch = NT * tau + tl
                u, jp = tch // 2, tch % 2
                if jp == 0:
                    ek8 = zp_kv.tile([128, 2 * D], F8, tag="ek", bufs=9,
                                     name=f"ek8_{u}")
                    ek8_t[u] = ek8
                    vt8 = zp_kv.tile([128, 2 * H * 65], F8, tag="vt", bufs=9,
                                     name=f"vt8_{u}")
                    vt8_t[u] = vt8
                    vt8_v = vt8[:].rearrange("p (j q c) -> p j q c", j=2, c=65)
                    nc.vector.memset(vt8_v[:, :, :, 64:65], 1.0)
                ek8, vt8 = ek8_t[u], vt8_t[u]
                vt8_v = vt8[:].rearrange("p (j q c) -> p j q c", j=2, c=65)

                pkv = pep.tile([128, 2 * D], F8, tag="pkv", name="pkv")
                nc.sync.dma_start(
                    pkv[:], d_pkv64[128 * tch:128 * (tch + 1), :])

                for jkv in range(4):  # k0 k1 v0 v1
                    ps = ps_mm.tile([128, TJ], F32, tag="mm", name="pskv")
                    for i in range(NP):
                        zv = z8_t[(i, tau)][:].rearrange(
                            "p (j n) -> p j n", n=TJ)[
                                :, :, 128 * tl:128 * (tl + 1)]
                        wv = wkv8_t[i][:].rearrange(
                            "p (j n) -> p j n", n=2 * D)[
                                :, :, TJ * jkv:TJ * (jkv + 1)]
                        nc.tensor.matmul(ps[:], zv, wv, start=(i == 0),
                                         stop=(i == NP - 1), perf_mode=DR)
                    tmp = work.tile([128, TJ], BF, tag="tmp", bufs=6,
                                    name="kvt")
                    nc.vector.tensor_tensor(
                        tmp[:], ps[:], pkv[:, TJ * jkv:TJ * (jkv + 1)],
                        ALU.add)
                    if jkv < 2:
                        nc.scalar.activation(
                            ek8[:, D * jp + TJ * jkv:D * jp + TJ * (jkv + 1)],
                            tmp[:], AF.Exp, scale=IWS)
                    else:
                        jv = jkv - 2
                        tmpv = tmp[:].rearrange("p (q c) -> p q c", c=64)
                        nc.scalar.activation(
                            vt8_v[:, jp, 8 * jv:8 * (jv + 1), 0:64], tmpv,
                            AF.Copy, scale=IWS)
            if tau == 1:
                emit_ctx_round(0)
        emit_ctx_round(1)

        # ======== pairwise AllReduce of ctx partials (+Zk columns) ========
        ar_in = dram.tile([128, 8 * 65], F32, tag="ar_in", name="ar_in")
        ar_out = dram.tile([128, 8 * 65], F32, tag="ar_out", name="ar_out")
        nc.sync.dma_start(ar_in[:], ctx_all[:])
        nc.gpsimd.collective_compute(
            "AllReduce", ALU.add,
            replica_groups=[[0, 1], [2, 3], [4, 5], [6, 7]],
            ins=[ar_in.opt()], outs=[ar_out.opt()])

        close_pool("zp_kv")
        zp_eq = open_pool("zp_eq", side="right")

        # ======== Phase C: q projection (fp8, unnormalized) + Zq ========
        def q_proj(tau):
            pqt = pep.tile([128, ND * TJ], F8, tag="pqt", bufs=2, name="pqt")
            nc.sync.dma_start(
                pqt[:], d_pq64[:, ND * TJ * tau:ND * TJ * (tau + 1)])
            for i in range(NP):
                eq8 = zp_eq.tile([128, 2 * TJ], F8, tag=f"eq_{i}_{tau}",
                                 name=f"eq_{i}_{tau}")
                eq8_t[(i, tau)] = eq8
                for j2 in range(2):
                    jq = 2 * i + j2
                    ps = ps_mm.tile([128, TJ], F32, tag="mm", name="psq")
                    for ip in range(NP):
                        wv = wq8_t[ip][:].rearrange(
                            "p (j n) -> p j n", n=D)[
                                :, :, 128 * jq:128 * (jq + 1)]
                        zv = z8_t[(ip, tau)][:].rearrange(
                            "p (j n) -> p j n", n=TJ)
                        nc.tensor.matmul(ps[:], wv, zv, start=(ip == 0),
                                         stop=(ip == NP - 1), perf_mode=DR)
                    tmp = work.tile([128, TJ], BF, tag="tmp", bufs=6,
                                    name="qt")
                    nc.vector.tensor_tensor(
                        tmp[:], ps[:], pqt[:, TJ * jq:TJ * (jq + 1)], ALU.add)
                    nc.scalar.activation(
                        eq8[:, TJ * j2:TJ * (j2 + 1)], tmp[:], AF.Exp,
                        scale=IWS)

        def q_stats(tau):
            pz = ps_st.tile([H, TJ], F32, tag="stat", name="pz")
            for i in range(NP):
                mv = mask8_t[i][:].rearrange("p (j n) -> p j n", n=H)
                ev = eq8_t[(i, tau)][:].rearrange("p (j n) -> p j n", n=TJ)
                nc.tensor.matmul(pz[:], mv, ev, start=(i == 0),
                                 stop=(i == NP - 1), perf_mode=DR)
            zq_f = rot.tile([H, TJ], F32, tag="zq_f", bufs=2, name="zq_f")
            nc.vector.reciprocal_approx_fast(zq_f[:], pz[:])
            zqb = zp_eq.tile([H, TJ], BF, tag=f"zqb{tau}", name=f"zqb{tau}")
            zqb_t[tau] = zqb
            nc.vector.tensor_copy(zqb[:], zq_f[:])

        q_proj(0)
        # third DMA wave: wow2 reuses the wkv8 buffers (free after the ctx
        # rounds); issued early so phase D never waits on weights
        wow2_t, w18_t = [], []
        for u in range(NP):
            wt = wpool.tile([128, 2 * 2 * D], F8, tag=f"wbig{u}",
                            name=f"wow2_{u}")
            nc.sync.dma_start(
                wt[:], d_wow28[:, 2 * 2 * D * u:2 * 2 * D * (u + 1)])
            wow2_t.append(wt)
        q_proj(1)
        q_stats(0)
        q_proj(2)
        q_stats(1)
        q_proj(3)
        for u in range(NP):
            wt = wpool.tile([128, 2 * D], F8, tag=f"wsm{u}", name=f"w18_{u}")
            nc.sync.dma_start(wt[:], d_w18[:, 2 * D * u:2 * D * (u + 1)])
            w18_t.append(wt)
        q_stats(2)
        q_stats(3)

        # ======== Phase D setup: AR unpack + scaled per-head ctx ========
        close_pool("zp_z")
        pdf = open_pool("pdf", side="right")

        ar_sb = zp_eq.tile([128, 8 * 65], F32, tag="ar_sb", name="ar_sb")
        nc.sync.dma_start(ar_sb[:], ar_out[:])
        ar_v = ar_sb[:].rearrange("p (i c) -> p i c", c=65)
        inv_zk = rot.tile([128, 8], F32, tag="inv_zk", name="inv_zk")
        nc.vector.reciprocal_approx_fast(inv_zk[:], ar_v[:, :, 64])
        # two heads per stationary tile: col block par holds head 2i+par,
        # nonzero only in its own 64 d-rows, so one [128,128] stationary
        # computes both heads' pa halves in a single matmul.
        ctxs2_t = []
        for i in range(8):
            cs = zp_eq.tile([128, 128], F8, tag=f"ctxs{i}", name=f"ctxs{i}")
            nc.vector.memset(cs[:], 0.0)
            for par in range(2):
                nc.vector.tensor_scalar(
                    cs[64 * par:64 * (par + 1),
                       64 * par:64 * (par + 1)],
                    ar_v[64 * par:64 * (par + 1), i, 0:64],
                    inv_zk[64 * par:64 * (par + 1), i:i + 1],
                    8.0, ALU.mult, ALU.mult)
            ctxs2_t.append(cs)

        # (phase D/E/F bias/rank-1 constants were DMA'd at kernel start)

        # ======== Phase D/E/F: software-pipelined over token tiles ========
        at8_tiles = {}

        def attn_ph(tau):
            lst = []
            for u in range(NP):
                at8 = pdf.tile([128, 2 * TJ], F8, tag="attn", bufs=9,
                               name=f"at8_{u}")
                lst.append(at8)
            at8_tiles[tau] = lst
            for i in range(ND):  # head pairs
                pa = ps_mm.tile([128, TJ], F32, tag="mm", name="pa")
                eqs = eq8_t[(i // 2, tau)][:, TJ * (i % 2):TJ * (i % 2 + 1)]
                nc.tensor.matmul(pa[:], ctxs2_t[i][:], eqs,
                                 start=True, stop=True)
                pb = ps_mm.tile([128, TJ], F32, tag="mm", name="pb")
                nc.tensor.matmul(pb[:], sel2_t[:, 128 * i:128 * (i + 1)],
                                 zqb_t[tau][:], start=True, stop=True)
                pbs = work.tile([128, TJ], BF, tag="pbs", bufs=3, name="pbs")
                nc.vector.tensor_copy(pbs[:], pb[:])
                nc.vector.tensor_tensor(
                    lst[i // 2][:, TJ * (i % 2):TJ * (i % 2 + 1)],
                    pa[:], pbs[:], ALU.mult)

        def wo_ph(tau, o_range=None):
            for o in (o_range if o_range is not None else range(ND)):
                ps = ps_mm.tile([128, TJ], F32, tag="mm", name="pswo")
                for u in range(NP):
                    wv = wow2_t[u][:].rearrange(
                        "p (j n) -> p j n", n=2 * D)[
                            :, :, 128 * o:128 * (o + 1)]
                    av = at8_tiles[tau][u][:].rearrange(
                        "p (j n) -> p j n", n=TJ)
                    nc.tensor.matmul(ps[:], wv, av, start=(u == 0),
                                     stop=False, perf_mode=DR)
                nc.tensor.matmul(ps[:], I4k, xbf_sl(o, tau),
                                 start=False, stop=True)
                x2 = pdf.tile([128, TJ], BF, tag="x2", bufs=13, name=f"x2_{o}")
                x2_t[(o, tau)] = x2
                nc.scalar.activation(x2[:], ps[:], AF.Identity,
                                     scale=1.0 / 4096.0,
                                     bias=boc_t[:, o:o + 1])

        def ln2_stats(tau):
            pm = ps_st.tile([1, TJ], F32, tag="stat", name="pm")
            pq2 = ps_st.tile([1, TJ], F32, tag="stat", name="pq2")
            for k in range(ND):
                sq = work.tile([128, TJ], BF, tag="lnsq", name=f"sq{k}")
                nc.vector.tensor_tensor(sq[:], x2_t[(k, tau)][:],
                                        x2_t[(k, tau)][:], ALU.mult)
                nc.tensor.matmul(pm[:], oneK[:], x2_t[(k, tau)][:],
                                 start=(k == 0), stop=(k == ND - 1))
                nc.tensor.matmul(pq2[:], oneK[:], sq[:],
                                 start=(k == 0), stop=(k == ND - 1))
            return pm, pq2

        def ln2_norm(tau, pm, pq2):
            m_sb = rot.tile([1, TJ], F32, tag="m_sb", name="m_sb")
            nc.scalar.copy(m_sb[:], pm[:])
            msq = rot.tile([1, TJ], F32, tag="msq", name="msq")
            nc.vector.tensor_tensor(msq[:], m_sb[:], m_sb[:], ALU.mult)
            var = rot.tile([1, TJ], F32, tag="var", name="var")
            nc.vector.tensor_tensor(var[:], pq2[:], msq[:], ALU.subtract)
            std = rot.tile([1, TJ], F32, tag="std", name="std")
            nc.scalar.activation(std[:], var[:], AF.Sqrt, bias=eps_col[0:1, :])
            rf = rot.tile([1, TJ], F32, tag="rf", name="rf")
            nc.vector.reciprocal_approx_fast(rf[:], std[:])
            r_bf = rot.tile([1, TJ], BF, tag="r_bf", bufs=2, name="r_bf")
            nc.vector.tensor_copy(r_bf[:], rf[:])
            mr_bf = rot.tile([1, TJ], BF, tag="mr_bf", bufs=2, name="mr_bf")
            nc.vector.tensor_tensor(mr_bf[:], m_sb[:], rf[:], ALU.mult)
            p_r2 = ps_mm.tile([128, TJ], F32, tag="mm", name="p_r2")
            nc.tensor.matmul(p_r2[:], one_row[:], r_bf[:], start=True,
                             stop=True)
            rbc2 = work.tile([128, TJ], BF, tag="rbc", bufs=2, name="rbc2")
            nc.scalar.copy(rbc2[:], p_r2[:])
            z2_l = []
            for i in range(NP):
                z2 = pdf.tile([128, 2 * TJ], F8, tag="z2", bufs=5,
                              name=f"z2_{i}")
                for j2 in range(2):
                    nc.vector.tensor_tensor(
                        z2[:, TJ * j2:TJ * (j2 + 1)],
                        x2_t[(2 * i + j2, tau)][:], rbc2[:], ALU.mult)
                z2_l.append(z2)
            return z2_l, mr_bf

        def mlp_ph(tau, z2_l, mr_bf):
            g_l = []
            for j in range(ND):
                if j % 2 == 0:
                    gt = pdf.tile([128, 2 * TJ], F8, tag="g", bufs=5,
                                  name=f"g{j // 2}")
                    g_l.append(gt)
                ps = ps_mm.tile([128, TJ], F32, tag="mm", name="psw1")
                for i in range(NP):
                    wv = w18_t[i][:].rearrange(
                        "p (j n) -> p j n", n=D)[:, :, 128 * j:128 * (j + 1)]
                    zv = z2_l[i][:].rearrange("p (j n) -> p j n", n=TJ)
                    nc.tensor.matmul(ps[:], wv, zv, start=(i == 0),
                                     stop=False, perf_mode=DR)
                nc.tensor.matmul(ps[:], csw1n_t[0:1, 128 * j:128 * (j + 1)],
                                 mr_bf[:], start=False, stop=True)
                nc.scalar.activation(
                    g_l[j // 2][:, TJ * (j % 2):TJ * (j % 2 + 1)], ps[:],
                    AF.Gelu, scale=IWS, bias=b1c_t[:, j:j + 1])
            for o in range(ND):
                ps = ps_mm.tile([128, TJ], F32, tag="mm", name="psw2")
                for u in range(NP):
                    wv = wow2_t[u][:].rearrange(
                        "p (j n) -> p j n", n=2 * D)[
                            :, :, D + 128 * o:D + 128 * (o + 1)]
                    gv = g_l[u][:].rearrange("p (j n) -> p j n", n=TJ)
                    nc.tensor.matmul(ps[:], wv, gv, start=(u == 0),
                                     stop=False, perf_mode=DR)
                nc.tensor.matmul(ps[:], I64, x2_t[(o, tau)][:], start=False,
                                 stop=True)
                of = work.tile([128, TJ], F32, tag="of1", bufs=3, name="of")
                nc.scalar.activation(of[:], ps[:], AF.Identity, scale=IWS,
                                     bias=b2c_t[:, o:o + 1])
                nc.sync.dma_start(
                    d_out[128 * o:128 * (o + 1), TJ * tau:TJ * (tau + 1)],
                    of[:])

        # pipeline: attn(t+1) + half of wo(t+1) fill the PE while LN2(t)'s
        # small-op chain runs on DVE/ACT
        attn_ph(0)
        wo_ph(0)
        for tau in range(NT):
            pm, pq2 = ln2_stats(tau)
            if tau + 1 < NT:
                attn_ph(tau + 1)
                wo_ph(tau + 1, range(0, 4))
            z2_l, mr_bf = ln2_norm(tau, pm, pq2)
            mlp_ph(tau, z2_l, mr_bf)
            if tau + 1 < NT:
                wo_ph(tau + 1, range(4, ND))

        close_pool("pdf")
        close_pool("zp_eq")

    nc.finalize()
    return nc


_CACHE = {}


def _get_nc():
    if "nc" not in _CACHE:
        import time
        t0 = time.time()
        _CACHE["nc"] = build_nc()
        print(f"[kernel] build_nc took {time.time() - t0:.1f}s", flush=True)
    return _CACHE["nc"]


def _host_prep(x, ln1_g, ln1_b, Wqkv, Wo, bo, ln2_g, ln2_b, W1, b1, W2, b2):
    x = np.asarray(x, np.float32)
    Wqkv = np.asarray(Wqkv, np.float32)

    pos = np.arange(S, dtype=np.float32)[:, None]
    div = np.exp(np.arange(0, D, 2, dtype=np.float32) * (-math.log(10000.0) / D))
    pe = np.zeros((S, D), dtype=np.float32)
    pe[:, 0::2] = np.sin(pos * div)
    pe[:, 1::2] = np.cos(pos * div)

    Wqkv_eff = np.asarray(ln1_g, np.float32)[:, None] * Wqkv
    wq8 = _pack_pairs(WS * Wqkv_eff[:, :D], D)
    wkv8 = _pack_pairs(WS * Wqkv_eff[:, D:], 2 * D)
    peW = (pe @ Wqkv + np.asarray(ln1_b, np.float32) @ Wqkv).astype(np.float32)
    csW = Wqkv_eff.sum(axis=0)                                   # [3D]

    xflat = x.reshape(B * S, D)
    m_all = xflat.mean(axis=1)
    var_all = xflat.var(axis=1)
    r_all = 1.0 / np.sqrt(var_all + EPS)
    mr_all = m_all * r_all

    W1_eff = np.asarray(ln2_g, np.float32)[:, None] * np.asarray(W1, np.float32)
    b1_eff = (np.asarray(b1, np.float32)
              + np.asarray(ln2_b, np.float32) @ np.asarray(W1, np.float32))

    wow28 = np.concatenate(
        [np.asarray(Wo, np.float32).reshape(NP, 2, 128, D),
         np.asarray(W2, np.float32).reshape(NP, 2, 128, D)],
        axis=3).reshape(D, 2 * D)
    wow28 = _pack_pairs(WS * wow28.reshape(D, 2 * D), 2 * D)
    w18 = _pack_pairs(WS * W1_eff, D)

    boc = np.ascontiguousarray(
        np.asarray(bo, np.float32).reshape(ND, 128).T).astype(np.float32)
    b1c = np.ascontiguousarray(b1_eff.reshape(ND, 128).T).astype(np.float32)
    b2c = np.ascontiguousarray(
        np.asarray(b2, np.float32).reshape(ND, 128).T).astype(np.float32)
    csw1n = _bf(-WS * W1_eff.sum(axis=0)[None, :])

    # mask: pair-packed [128, NP*2*H] one-hot head selector (fp8)
    mask = np.zeros((NP, 2, 128, H), dtype=np.float32)
    for i in range(NP):
        for j in range(2):
            rows = np.arange(128) + 128 * (2 * i + j)
            mask[i, j, np.arange(128), rows // DH] = 1.0
    mask8 = _f8(mask.transpose(2, 0, 1, 3).reshape(128, NP * 2 * H))

    sel2 = np.zeros((H, D), dtype=np.float32)
    cols = np.arange(D)
    sel2[2 * (cols // 128) + (cols % 128) // 64, cols] = 1.0
    sel2 = _bf(sel2)

    identd = np.concatenate(
        [np.eye(128, dtype=np.float32),
         4096.0 * np.eye(128, dtype=np.float32),
         WS * np.eye(128, dtype=np.float32)], axis=1)
    identd = _bf(identd)

    in_maps = []
    for c in range(N_CORES):
        toks = slice(c * T, (c + 1) * T)
        pos0 = (c % 2) * T
        posr = slice(pos0, pos0 + T)
        mr_c = mr_all[toks][:, None]                              # [T, 1]
        pkv64 = WS * (peW[posr, D:] - mr_c * csW[None, D:])       # [T, 2D]
        pq64 = WS * (peW[posr, :D] - mr_c * csW[None, :D])       # [T, D]
        # pack per-tau: [128, tau*(ND*TJ) + jq*TJ + n] = pq64[TJ*tau+n,
        # 128*jq+p]
        pq64p = pq64.reshape(NT, TJ, ND, 128).transpose(3, 0, 2, 1).reshape(
            128, NT * ND * TJ)
        in_maps.append({
            "xbf": _bf(xflat[toks].T),
            "r1_row": _bf(r_all[toks][None, :]),
            "wq8": wq8, "wkv8": wkv8, "wow28": wow28, "w18": w18,
            "pkv64": _f8(pkv64),
            "pq64p": _f8(pq64p),
            "identd": identd,
            "mask8": mask8, "sel2": sel2, "csw1n": csw1n,
            "boc": boc, "b1c": b1c, "b2c": b2c,
        })
    return in_maps


def run(inputs: dict, trace: bool = False):
    nc = _get_nc()
    in_maps = _host_prep(**inputs)
    res = run_bass_kernel_spmd(nc, in_maps, core_ids=list(range(N_CORES)),
                               trace=trace)
    outs = [res.results[c]["outT"] for c in range(N_CORES)]
    full = np.concatenate([o.T for o in outs], axis=0).reshape(B, S, D)
    return full.astype(np.float32), res


def kernel(**inputs) -> np.ndarray:
    out, _ = run(inputs, trace=False)
    return out
